# revision 15
# baseline (speedup 1.0000x reference)
"""BERT self-attention kernel for Trainium2, sharded over 8 NeuronCores.

Problem: nn_CustomBertSelfAttention (B=2, S=2048, D=1024, H=16 heads, HD=64).

Sharding: tensor-parallel over heads. Core c owns heads {2c, 2c+1}, i.e.
columns [128c, 128c+128) of Wq/Wk/Wv and of the output. Every core reads the
full hidden_states (transposed + cast to bf16 on the host so the contraction
dim lands on SBUF partitions with dense DMA).

Per-core pipeline (all matmuls bf16 with f32 PSUM accumulation):
  1. Projections Q^T/K^T/V^T [128, BS] = W^T @ x^T as six uniform sections
     (q/k/v x batch) on a ring of psum tiles — no pool barriers. Q/K get
     their bias on DVE during eviction; the V bias is applied on the host.
     V^T is PE-transposed back to V [keys, hd] and stored interleaved:
     vv[b] = [V_h0(64) | 1 | V_h1(64) | 1] per key tile, so each unit's
     augmented stationary [V|1] is one contiguous 65-column slice.
  2. Attention per unit (b, h), key-tile-outer with both 1024-wide query
     halves as lanes: scores^T [keys, q] = K_tile^T.T @ Q^T for lane 0 and 1
     (4 matmuls sharing one stationary load), exp on ScalarE with the
     additive attention mask folded in as the activation's per-partition
     bias (exact: exp(s*sc + m) = e^m e^{s*sc}), then
     ctx^T [65, q] += [V|1]^T @ P^T for both lanes (again one stationary
     load), accumulated over key tiles. Row 64 is the softmax denominator.
     No on-device normalization: the raw [65, S] goes to DRAM and the host
     divides (and adds the V bias).
  3. A post-build IR pass drops InstLdweights whose stationary is identical
     to the one already loaded, removing redundant ~100ns PE weight reloads
     the tile framework emits per matmul. The tile scheduler overlaps the
     projection tail with early attention on its own.
Host: out[u] = (ctx[0:64] / ctx[64])^T + bv  gathered into [B, S, D].
"""
import sys

sys.path.insert(0, "/opt/trn_rl_repo")

import numpy as np
import ml_dtypes

from concourse import bacc
import concourse.mybir as mybir
from concourse.tile import TileContext
from concourse.masks import make_identity
from concourse.bass_utils import run_bass_kernel_spmd

B, S, D, H, HD = 2, 2048, 1024, 16, 64
N_CORES = 8
HPC = H // N_CORES          # heads per core = 2
DC = D // N_CORES           # output/weight columns per core = 128
BS = B * S                  # 4096
NU = B * HPC                # attention units per core = 4
P = 128
F32 = mybir.dt.float32
BF16 = mybir.dt.bfloat16
KT = S // P                 # 16 key tiles per unit
QH = 1024                   # query lane width
NL = S // QH                # 2 query lanes per unit
SCH = 1024                  # projection chunk (BS columns per psum tile)
W65 = HD + 1                # V_aug width (V columns + ones column)
W130 = 2 * W65              # two heads interleaved per key tile in vv[b]
DT = D // P                 # 8 contraction tiles
SCALE = float(1.0 / np.sqrt(HD))

DEDUPE_LDWEIGHTS = True

_cached_nc = None


def _ap_key(arg):
    """Stable identity key for an LDWEIGHTS stationary access pattern."""
    try:
        bass_ap = getattr(arg, "bass_ap", None)
        if bass_ap is not None:
            return ("bap", bass_ap.tensor.name, bass_ap.offset,
                    tuple(map(tuple, bass_ap.ap)), str(arg.dtype))
        return ("raw", getattr(arg, "memref", ""), arg.offset,
                tuple(map(tuple, arg.ap)), str(arg.dtype))
    except Exception:
        return ("repr", repr(arg))


def _dedupe_ldweights(nc):
    """Drop PE weight reloads whose stationary is already in the array.

    The tile legalizer splits every InstMatmult into InstLdweights +
    InstMatmult. Runs of matmuls that share a stationary reload it
    redundantly; the PE array retains the stationary across matmuls, so
    duplicate loads are pure overhead (~100ns each). Dependencies carried
    by a dropped load are merged into the next PE instruction so no
    synchronization is lost. Operates on the post-scheduler order, so only
    loads that are genuinely redundant at execution time are removed.
    """
    pe = mybir.EngineType.PE
    for f in nc.m.functions:
        for blk in f.blocks:
            insts = blk.instructions
            drop = set()
            cur_key = None
            pending_merge = []  # deps from dropped LDs awaiting next PE inst
            for i in insts:
                if getattr(i, "engine", None) != pe:
                    continue
                tn = type(i).__name__
                if tn == "InstLdweights":
                    key = (
                        _ap_key(i.ins[0]),
                        getattr(i, "is_transpose", None),
                        getattr(i, "perf_mode", None),
                        getattr(i, "tile_position", None),
                    )
                    if key == cur_key:
                        drop.add(id(i))
                        pending_merge.append(i)
                    else:
                        cur_key = key
                elif pending_merge:
                    for ld in pending_merge:
                        i.merge_dependencies_from(ld)
                    pending_merge = []
            if drop:
                blk.instructions = [i for i in insts if id(i) not in drop]


def _mm_pair(nc, ps, lhsT, rhs0, rhs1, start, stop):
    """Two n=512 matmuls sharing one stationary (reload deduped later)."""
    nc.tensor.matmul(ps[:, 0:512], lhsT=lhsT, rhs=rhs0, start=start, stop=stop)
    nc.tensor.matmul(ps[:, 512:1024], lhsT=lhsT, rhs=rhs1, start=start,
                     stop=stop)


def build_nc():
    nc = bacc.Bacc(None, target_bir_lowering=False)

    xT = nc.dram_tensor("xT", [D, BS], BF16, kind="ExternalInput")
    # weights host-pre-tiled to [P, DT*DC] so the DMA is a plain 2D copy
    w_in = {
        pr: nc.dram_tensor(f"w{pr}", [P, DT * DC], BF16, kind="ExternalInput")
        for pr in "qkv"
    }
    bqkv = nc.dram_tensor("bqkv", [DC, 3], F32, kind="ExternalInput")
    # mask host-pre-tiled to [P, B*KT] (key position on partitions)
    mkT = nc.dram_tensor("mkT", [P, B * KT], F32, kind="ExternalInput")
    out = nc.dram_tensor("out", [NU, W65, S], F32, kind="ExternalOutput")

    from contextlib import ExitStack

    with TileContext(nc) as tc, ExitStack() as es:
        const = es.enter_context(tc.tile_pool(name="const", bufs=1))
        wp = es.enter_context(tc.tile_pool(name="wsb", bufs=1))
        qkvp = es.enter_context(tc.tile_pool(name="qkv", bufs=1))
        xp = es.enter_context(tc.tile_pool(name="xsb", bufs=1))
        ptp = es.enter_context(tc.tile_pool(name="pt", bufs=4))
        obp = es.enter_context(tc.tile_pool(name="ob", bufs=2))

        ident = const.tile([P, P], BF16)
        make_identity(nc, ident)
        b_sb = const.tile([DC, 3], F32)
        mk = const.tile([P, B * KT], F32)
        w_sb = {
            pr: wp.tile([P, DT * DC], BF16, tag=f"w{pr}", name=f"w{pr}sb")
            for pr in "qkv"
        }
        # x^T staged in SBUF: one tile per (d-tile, batch, 1024-col half)
        xx = {}
        for b in range(B):
            for dt in range(DT):
                for h in range(2):
                    xx[(dt, b, h)] = xp.tile(
                        [P, SCH], BF16, tag=f"x{dt}_{b}_{h}",
                        name=f"x{dt}_{b}_{h}")

        def x_dma(dt, b, h):
            c0 = b * S + h * SCH
            nc.sync.dma_start(
                xx[(dt, b, h)][:], xT[dt * P:(dt + 1) * P, c0:c0 + SCH]
            )

        # DMA order matters: the sync queue serializes configs (~0.6us
        # each), so emit exactly what the first section needs first.
        nc.sync.dma_start(w_sb["q"][:], w_in["q"][:])
        for dt in range(DT):
            x_dma(dt, 0, 0)
            x_dma(dt, 0, 1)
        nc.sync.dma_start(w_sb["k"][:], w_in["k"][:])
        nc.sync.dma_start(w_sb["v"][:], w_in["v"][:])
        nc.sync.dma_start(b_sb[:], bqkv[:])
        for dt in range(DT):
            x_dma(dt, 1, 0)
            x_dma(dt, 1, 1)
        nc.sync.dma_start(mk[:], mkT[:])

        # Persistent per-core activations
        q_sb = qkvp.tile([P, BS], BF16)       # Q^T: [dq, (b s)]
        k_sb = qkvp.tile([P, BS], BF16)       # K^T
        v_t = qkvp.tile([P, BS], BF16)        # V^T staging (pre-transpose)
        vv = [
            qkvp.tile([P, KT * W130], BF16, tag=f"vv{b}", name=f"vv{b}")
            for b in range(B)
        ]

        def ones_memset(b):
            view = vv[b][:].rearrange("p (t g w) -> p t g w", g=2, w=W65)
            nc.vector.memset(view[:, :, :, W65 - 1:W65].squeeze(-1), 1.0)

        # ---------------- Phase A: projections + V layout ----------------
        with nc.named_scope("proj"):
            with tc.tile_pool(name="pp", bufs=3, space="PSUM") as pp, \
                 tc.tile_pool(name="pT", bufs=2, space="PSUM") as pT:

                def section(pr, b):
                    # both 1024-col chunks of (pr, batch b); dt-outer so the
                    # two chunks share each stationary load
                    ps = [
                        pp.tile([P, SCH], F32, tag="pp", name="pp")
                        for _ in range(2)
                    ]
                    for dt in range(DT):
                        for h in range(2):
                            _mm_pair(
                                nc, ps[h],
                                w_sb[pr][:, dt * DC:(dt + 1) * DC],
                                xx[(dt, b, h)][:, 0:512],
                                xx[(dt, b, h)][:, 512:SCH],
                                start=(dt == 0), stop=(dt == DT - 1),
                            )
                    for h in range(2):
                        sl = slice(b * S + h * SCH, b * S + (h + 1) * SCH)
                        if pr == "q":
                            nc.vector.tensor_scalar_add(
                                q_sb[:, sl], ps[h][:], b_sb[:, 0:1])
                        elif pr == "k":
                            nc.vector.tensor_scalar_add(
                                k_sb[:, sl], ps[h][:], b_sb[:, 1:2])
                        else:
                            nc.vector.tensor_copy(v_t[:, sl], ps[h][:])

                def vt_transposes(b):
                    for kt in range(KT):
                        tp = pT.tile([P, P], BF16, tag="tp", name="tp")
                        nc.tensor.transpose(
                            tp[:],
                            v_t[:, b * S + kt * P:b * S + (kt + 1) * P],
                            ident[:],
                        )
                        dst = vv[b][:, kt * W130:(kt + 1) * W130].rearrange(
                            "p (g w) -> p g w", w=W65)
                        nc.vector.tensor_copy(
                            dst[:, :, 0:HD],
                            tp[:].rearrange("p (g d) -> p g d", d=HD),
                        )

                ones_memset(0)
                ones_memset(1)
                for pr in "qkv":
                    section(pr, 0)
                vt_transposes(0)
                for pr in "qkv":
                    section(pr, 1)
                vt_transposes(1)

        # ---------------- Phase B: attention ----------------
        with nc.named_scope("attn"):
            with tc.tile_pool(name="sps", bufs=2, space="PSUM") as sp, \
                 tc.tile_pool(name="cps0", bufs=1, space="PSUM") as cp0, \
                 tc.tile_pool(name="cps1", bufs=1, space="PSUM") as cp1:
                cpools = [cp0, cp1]

                for u in range(NU):
                    b, hl = u // HPC, u % HPC
                    hp = slice(hl * HD, (hl + 1) * HD)
                    bs0 = b * S
                    cps = [
                        cpools[l].tile([W65, QH], F32, tag=f"cps{l}",
                                       name=f"cps{l}")
                        for l in range(NL)
                    ]
                    pts = [[None] * NL for _ in range(KT)]

                    def emit_ctx(j, u=u, b=b, hl=hl, cps=cps, pts=pts):
                        o0 = j * W130 + hl * W65
                        for l in range(NL):
                            _mm_pair(
                                nc, cps[l],
                                vv[b][:, o0:o0 + W65],
                                pts[j][l][:, 0:512],
                                pts[j][l][:, 512:1024],
                                start=(j == 0), stop=(j == KT - 1),
                            )

                    for kt in range(KT):
                        sps = [sp.tile([P, QH], F32, tag="sps", name="sps")
                               for _ in range(NL)]
                        for l in range(NL):
                            q0 = bs0 + l * QH
                            _mm_pair(
                                nc, sps[l],
                                k_sb[hp, bs0 + kt * P:bs0 + (kt + 1) * P],
                                q_sb[hp, q0:q0 + 512],
                                q_sb[hp, q0 + 512:q0 + QH],
                                start=True, stop=True,
                            )
                        if kt > 0:
                            emit_ctx(kt - 1)
                        for l in range(NL):
                            pt = ptp.tile([P, QH], BF16, tag="pt")
                            nc.scalar.activation(
                                pt[:], sps[l][:],
                                mybir.ActivationFunctionType.Exp,
                                bias=mk[:, b * KT + kt:b * KT + kt + 1],
                                scale=SCALE,
                            )
                            pts[kt][l] = pt
                    emit_ctx(KT - 1)
                    for l in range(NL):
                        ob = obp.tile([W65, QH], F32, tag="ob")
                        nc.vector.tensor_copy(ob[:], cps[l][:])
                        nc.sync.dma_start(
                            out[u, :, l * QH:(l + 1) * QH], ob[:])

    if DEDUPE_LDWEIGHTS:
        _dedupe_ldweights(nc)
    nc.compile()
    return nc


def _prep_in_maps(hidden_states, attention_mask, Wq, bq, Wk, bk, Wv, bv):
    bf = ml_dtypes.bfloat16
    hs = np.asarray(hidden_states, dtype=np.float32).reshape(BS, D)
    xT = np.ascontiguousarray(hs.T).astype(bf)
    # mask pre-tiled: mkT[p, b*KT + t] = mask[b, t*P + p]
    mkT = np.ascontiguousarray(
        np.asarray(attention_mask, dtype=np.float32).reshape(B, KT, P)
        .transpose(2, 0, 1).reshape(P, B * KT)
    )
    Ws = {"q": np.asarray(Wq, np.float32), "k": np.asarray(Wk, np.float32),
          "v": np.asarray(Wv, np.float32)}
    bs = {"q": np.asarray(bq, np.float32), "k": np.asarray(bk, np.float32),
          "v": np.asarray(bv, np.float32)}
    in_maps = []
    for c in range(N_CORES):
        sl = slice(c * DC, (c + 1) * DC)
        m = {"xT": xT, "mkT": mkT}
        for pr in "qkv":
            # pre-tiled: [P, DT*DC], column block dt = rows [dt*P,(dt+1)*P)
            wc = Ws[pr][:, sl].reshape(DT, P, DC).transpose(1, 0, 2)
            m[f"w{pr}"] = np.ascontiguousarray(wc.reshape(P, DT * DC)).astype(bf)
        m["bqkv"] = np.ascontiguousarray(
            np.stack([bs["q"][sl], bs["k"][sl], bs["v"][sl]], axis=1)
        )
        in_maps.append(m)
    return in_maps


def _gather(results, bv):
    bv = np.asarray(bv, np.float32)
    full = np.empty((B, S, D), dtype=np.float32)
    for c in range(N_CORES):
        o = results[c]["out"]  # [NU, 65, S] unnormalized ctx^T + denom row
        for b in range(B):
            for hl in range(HPC):
                u = b * HPC + hl
                col = c * DC + hl * HD
                ctx = o[u, :HD, :] / o[u, HD:HD + 1, :]
                full[b, :, col:col + HD] = ctx.T + bv[col:col + HD]
    return full


def kernel(hidden_states, attention_mask, Wq, bq, Wk, bk, Wv, bv, **run_kwargs):
    global _cached_nc
    if _cached_nc is None:
        _cached_nc = build_nc()
    in_maps = _prep_in_maps(
        hidden_states, attention_mask, Wq, bq, Wk, bk, Wv, bv
    )
    res = run_bass_kernel_spmd(
        _cached_nc, in_maps, core_ids=list(range(N_CORES)), **run_kwargs
    )
    full = _gather(res.results, bv)
    if run_kwargs:
        kernel.last_result = res
    return full
